# revision 8
# baseline (speedup 1.0000x reference)
"""Chamfer distance (nn_ChamferLossLayer) on 8 Trainium2 NeuronCores.

Strategy (sharding_hint: shard P1 rows across devices):
  - Each core gets a 1500-row shard of cloud1 (both batches) + full cloud2.
  - Squared distances D[j, i] = sq2_j + sq1_i - 2<c2_j, c1_i> are computed on
    the PE as an augmented K=24 bf16 matmul (3-way hi/mid/lo split of each
    operand; all product pairs >= 2^-27 kept, so D is fp32-accurate to ~1e-7):
    stationary = 128 cloud2 points, moving = the core's 1536-padded shard.
  - One custom DVE op per 128x1536 tile (single 1x pass over PSUM) computes
    BOTH reductions: out[:, :1536] = min(D, run_i) (elementwise running min
    over j-tiles -> i-side), out[:, 1536] = scan-min over the row (j-side
    per-j row min; PSUM col 1536 is memset to a huge pad once).
  - ScalarE copies each tile's row-min column into a per-j-tile slot; A/B
    alternating run_i buffers keep that copy off the DVE critical path.
  - Host: partition-min of run_i (i-side), cross-core min of j-side, means.
"""

import numpy as np
import ml_dtypes

import concourse.bacc as bacc
import concourse.mybir as mybir
import concourse.dve_ops as dve_ops
from concourse.dve_spec import (
    Spec, Src0, Src1, C0, C2, AluOp, Idx, minn, select, scan, lower, _has_src1,
)
from concourse.dve_uop import DveOpSpec
from concourse.bass_utils import run_bass_kernel_spmd
from concourse.tile import TileContext

F32 = mybir.dt.float32
BF16 = mybir.dt.bfloat16
MIN = mybir.AluOpType.min
BF = ml_dtypes.bfloat16

N_CORES = 8
N, P, D = 2, 12000, 3          # batches, points per cloud, dims
SHARD = P // N_CORES           # 1500 cloud1 rows per core
FDI = 1504                     # padded shard width (512+512+480 matmul chunks)
NJT = (P + 127) // 128         # 94 j-tiles of 128 cloud2 points (12032 padded)
PJ = NJT * 128                 # 12032
K = 24                         # augmented contraction dim (3-way hi/mid/lo split)
BIG = 60000.0                  # pad distance, >> max real squared distance


def _register_minmin_op():
    """Custom DVE op: out[k] = min(in0[k], in1[k]) for k < imm2,
    out[k] = running-min(in0[0..k]) for k >= imm2 (row min lands at the
    last element). s0 seeds the scan (pass +huge)."""
    name = "CHAMFER_MINMIN_ANT"
    for op in dve_ops.OPS:
        if op.name == name:
            return op
    body = select(Idx < C2, minn(Src0, Src1), scan(AluOp.MIN, Src0, init=C0))

    def ref(in0, in1, c0, c1, c2):
        idx = np.arange(in0.shape[-1])[None, :]
        run = np.minimum.accumulate(in0.astype(np.float32), axis=-1)
        run = np.minimum(run, np.float32(c0))
        return np.where(idx < c2, np.minimum(in0, in1), run).astype(np.float32)

    spec = Spec(body=body, reference=ref)
    row = 1 + len(dve_ops.OPS)
    assert row < 0x20
    shas = {}
    for ver in ("v3", "v4"):
        s = DveOpSpec(name=name, opcode=row, uops=lower(spec, ver=ver),
                      rd1_en=_has_src1(spec))
        shas[ver] = s.sha(ver)
    op = dve_ops.DveOp(name=name, spec=spec, subdim=False, uops_sha=shas)
    dve_ops.OPS.append(op)
    dve_ops.CUSTOM_DVE_SPECS[name] = spec
    dve_ops._SUB_OPCODE_FOR_NAME[name] = row
    return op


_NC = None


def _build_program():
    """One SPMD program, run identically on all 8 cores."""
    global _NC
    if _NC is not None:
        return _NC
    op = _register_minmin_op()
    nc = bacc.Bacc()
    v = nc.dram_tensor("v", [N, K, PJ], BF16, kind="ExternalInput")
    u = nc.dram_tensor("u", [N, K, FDI], BF16, kind="ExternalInput")
    imin = nc.dram_tensor("imin", [N, 128, FDI], F32, kind="ExternalOutput")
    jmin = nc.dram_tensor("jmin", [N, 128, NJT], F32, kind="ExternalOutput")

    with TileContext(nc) as tc:
        with tc.tile_pool(name="sbuf", bufs=1) as pool, \
             tc.tile_pool(name="psum", bufs=1, space="PSUM") as pp:
            ps = [pp.tile([128, FDI + 1], F32, name=f"ps{k}", tag=f"ps{k}")
                  for k in range(2)]
            for k in range(2):
                # pad col read by every scan; banks 0-2 hold matmul output
                nc.vector.memset(ps[k][:, FDI:FDI + 1], BIG)
            for n in range(N):
                v_sb = pool.tile([K, PJ], BF16, tag=f"v{n}")
                u_sb = pool.tile([K, FDI], BF16, tag=f"u{n}")
                # u + first v-chunk land first so matmuls start early
                nc.sync.dma_start(out=u_sb[:, :], in_=u[n, :, :])
                nc.sync.dma_start(out=v_sb[:, 0:1024], in_=v[n, :, 0:1024])
                nc.sync.dma_start(out=v_sb[:, 1024:], in_=v[n, :, 1024:])
                run = [pool.tile([128, FDI + 1], F32, name=f"run{n}{k}",
                                  tag=f"run{n}{k}") for k in range(2)]
                nc.gpsimd.memset(run[0][:, :], BIG)
                nc.gpsimd.memset(run[1][:, :], BIG)
                jm = pool.tile([128, NJT], F32, tag=f"jm{n}")
                for jt in range(NJT):
                    pk = ps[jt % 2]
                    rk = run[jt % 2]
                    for c0, cn in ((0, 512), (512, 512), (1024, 480)):
                        nc.tensor.matmul(
                            pk[:, c0:c0 + cn],
                            v_sb[:, 128 * jt:128 * (jt + 1)],
                            u_sb[:, c0:c0 + cn],
                            start=True, stop=True)
                    nc.vector._custom_dve(
                        op, out=rk[:, :], in0=pk[:, :], in1=rk[:, :],
                        s0=3.0e38, imm2=float(FDI))
                    nc.scalar.copy(jm[:, jt:jt + 1], rk[:, FDI:FDI + 1])
                nc.vector.tensor_tensor(run[0][:, :], run[0][:, :],
                                        run[1][:, :], MIN)
                nc.sync.dma_start(out=imin[n, :, :], in_=run[0][:, 0:FDI])
                nc.sync.dma_start(out=jmin[n, :, :], in_=jm[:, :])
    nc.finalize()
    _NC = nc
    return nc


def _split3(x):
    """3-way bf16 split: hi + mid + lo ~= x to ~2^-27 relative."""
    hi = x.astype(BF)
    r = x - hi.astype(np.float32)
    mid = r.astype(BF)
    lo = (r - mid.astype(np.float32)).astype(BF)
    return hi, mid, lo


def _host_prep(cloud1, cloud2):
    """Build augmented bf16 operands. V (stationary, cloud2): [N, K, PJ];
    U (moving, cloud1): [N, K, P] to be sharded per core.
    D[j, i] = sum_k V[k, j] * U[k, i] ~= sq2_j + sq1_i - 2 <c2_j, c1_i>."""
    c1 = np.asarray(cloud1, np.float32)
    c2 = np.asarray(cloud2, np.float32)
    c1hi, c1mid, c1lo = _split3(c1)    # [N, P, 3]
    c2hi, c2mid, c2lo = _split3(c2)
    sq1 = np.einsum("npd,npd->np", c1.astype(np.float64),
                    c1.astype(np.float64)).astype(np.float32)
    sq2 = np.einsum("npd,npd->np", c2.astype(np.float64),
                    c2.astype(np.float64)).astype(np.float32)
    sq1s = _split3(sq1)
    sq2s = _split3(sq2)

    big = np.float32(BIG)
    one = BF(1.0)

    def t(a):
        return a.transpose(0, 2, 1)

    # kept coordinate product pairs (V-part, U-part):
    # (hi,hi) (hi,mid) (hi,lo) (mid,hi) (mid,mid) (lo,hi)
    v_coord = [c2hi, c2hi, c2hi, c2mid, c2mid, c2lo]
    u_coord = [c1hi, c1mid, c1lo, c1hi, c1mid, c1hi]

    V = np.zeros((N, K, PJ), BF)
    for r, arr in enumerate(v_coord):
        V[:, 3 * r:3 * (r + 1), :P] = t(arr)
    for r in range(3):
        V[:, 18 + r, :P] = sq2s[r]     # sq2 3-way, pairs with U ones
        V[:, 21 + r, :] = one          # pairs with U sq1 3-way (pads too)
    # pad j's: D = BIG + sq1_i
    V[:, 18, P:] = BF(big)

    U = np.zeros((N, K, P), BF)
    for r, arr in enumerate(u_coord):
        U[:, 3 * r:3 * (r + 1)] = (-2.0 * t(arr).astype(np.float32)).astype(BF)
    for r in range(3):
        U[:, 18 + r] = one             # pairs with V sq2 3-way
        U[:, 21 + r] = sq1s[r]
    return V, U


def kernel(cloud1, cloud2):
    nc = _build_program()
    V, U = _host_prep(cloud1, cloud2)

    in_maps = []
    for c in range(N_CORES):
        u_c = np.zeros((N, K, FDI), BF)
        u_c[:, :, :SHARD] = U[:, :, SHARD * c:SHARD * (c + 1)]
        # pad i's: D = sq2_j + BIG
        u_c[:, 18:21, SHARD:] = BF(1.0)
        u_c[:, 21, SHARD:] = BF(np.float32(BIG))
        u_c[:, 22:24, SHARD:] = 0
        in_maps.append({"v": V, "u": u_c})

    br = run_bass_kernel_spmd(nc, in_maps, list(range(N_CORES)))

    # i-side: per-core run_i [N, 128, FDI]; min over the 128 j-lanes, then
    # concat shards and mean over the 12000 cloud1 points.
    imins = []
    for c in range(N_CORES):
        m = br.results[c]["imin"].min(axis=1)     # [N, FDI]
        imins.append(m[:, :SHARD])
    imin_full = np.concatenate(imins, axis=1)      # [N, 12000]
    term1 = imin_full.mean(axis=1)

    # j-side: per-core jmin [N, 128, NJT] over the core's i-shard; min
    # across cores, reorder to j = 128*jt + lane, drop j-pads, mean.
    jstack = np.stack([br.results[c]["jmin"] for c in range(N_CORES)])
    jmin_all = jstack.min(axis=0)                  # [N, 128, NJT]
    jmin_full = jmin_all.transpose(0, 2, 1).reshape(N, PJ)[:, :P]
    term2 = jmin_full.mean(axis=1)

    return (term1 + term2).astype(np.float32)
